# revision 1
# baseline (speedup 1.0000x reference)
"""BitLinear158 Trainium2 kernel (per-core body + host driver).

Per core: x_shard [M_LOC, K] bf16 -> per-token int8 quant -> bf16 matmul
against host-unpacked ternary wT [K, N] -> rescale -> y [M_LOC, N] bf16.

v2 pipeline (per chunk of 1024 tokens, software-pipelined one chunk ahead):
  quant (natural layout):
    x tile [128,2048]  <- sync-ring DMA
    amax = DVE reduce(abs max); s = bf16(127/amax); r = 1/s (f32)
    t = bf16(x*s)      <- ACT activation(Copy, scale=s)   [matches jax bf16 mul]
    xq8 = int8(t)      <- DVE tensor_copy (RNE + saturate == round + clip)
    xq_dram (bf16)     <- gpsimd SWDGE DMA-cast store (int8 -> bf16)
  transpose: xqT[kc] [128,1024] <- sync-ring xbar DMA transpose of xq_dram
  matmul:   PSUM [128m,512n] f32 += xqT[kc][:,mi].T @ wT[:,kc,nt] (16 k-chunks)
  rescale:  y_sb = ACT activation(Copy, scale=r) -> bf16 ; y <- scalar-ring DMA
"""

import sys

sys.path.insert(0, "/opt/trn_rl_repo")

from contextlib import ExitStack

import numpy as np
import ml_dtypes

import concourse.bass as bass
import concourse.tile as tile
from concourse import bacc, mybir
from concourse import bass_utils

P = 128
M_LOC = 4096      # tokens per core
K = 2048          # in features
N = 2048          # out features
KC = K // P       # 16 k-chunks
NT = M_LOC // P   # 32 m-tiles per core
# chunk sizes in m-tiles: small first chunk so the PE starts early
CHUNK_MTS = [2, 4, 4, 4, 4, 4, 4, 4, 2]
assert sum(CHUNK_MTS) == NT
CHUNK_STARTS = [sum(CHUNK_MTS[:i]) for i in range(len(CHUNK_MTS))]
CHUNKS = len(CHUNK_MTS)
MAX_CHUNK_MT = max(CHUNK_MTS)
N_TILE = 512
NTN = N // N_TILE                  # 4
N_CORES = 8

BF16 = mybir.dt.bfloat16
F32 = mybir.dt.float32
I8 = mybir.dt.int8

FUSED_QUANT = False  # if True: single ACT op does int8(round(x*s)) in f32 (skips bf16 intermediate)


def build_kernel(replays: int = 1, fused_quant: bool = FUSED_QUANT):
    nc = bacc.Bacc("TRN2", target_bir_lowering=False, debug=False, num_devices=N_CORES)
    x = nc.dram_tensor("x", [M_LOC, K], BF16, kind="ExternalInput").ap()
    wT = nc.dram_tensor("wT", [K, N], BF16, kind="ExternalInput").ap()
    y = nc.dram_tensor("y", [M_LOC, N], BF16, kind="ExternalOutput").ap()

    x_tiled = x.rearrange("(t p) k -> t p k", p=P)
    y_tiled = y.rearrange("(t p) n -> t p n", p=P)
    wT_tiled = wT.rearrange("(c p) n -> p c n", p=P)

    with tile.TileContext(nc) as tc, ExitStack() as ctx:
        wbuf = ctx.enter_context(tc.tile_pool(name="wbuf", bufs=1))
        xin = ctx.enter_context(tc.tile_pool(name="xin", bufs=5))
        qtmp = ctx.enter_context(tc.tile_pool(name="qtmp", bufs=3))
        xqT_pool = ctx.enter_context(tc.tile_pool(name="xqT", bufs=3))
        stat = ctx.enter_context(tc.tile_pool(name="stat", bufs=4))
        rbuf = ctx.enter_context(tc.tile_pool(name="rbuf", bufs=1))
        yout = ctx.enter_context(tc.tile_pool(name="yout", bufs=5))
        psum = ctx.enter_context(tc.tile_pool(name="psum", bufs=8, space="PSUM"))
        dram = ctx.enter_context(tc.tile_pool(name="dram", bufs=1, space="DRAM"))

        wt = wbuf.tile([P, KC, N], BF16)
        nc.scalar.dma_start(wt[:], wT_tiled)

        r_all = rbuf.tile([P, NT], F32)

        def chunk_of(mt):
            for c in range(CHUNKS):
                if mt < CHUNK_STARTS[c] + CHUNK_MTS[c]:
                    return c, mt - CHUNK_STARTS[c]
            raise AssertionError

        for rep in range(replays):
            xq_dram = [
                dram.tile(
                    [CHUNK_MTS[c] * P, K], BF16,
                    tag=f"xq_dram{c}", name=f"xq_dram{c}",
                )
                for c in range(CHUNKS)
            ]

            # x loads run on the gpsimd (SWDGE) queue with a 2-tile lookahead so
            # the quant chain never waits on a load and no HWDGE ring is touched.
            xt_tiles = {}

            def load_tile(mt):
                if mt >= NT or mt in xt_tiles:
                    return
                xt = xin.tile([P, K], BF16, tag="xt", name="xt")
                nc.gpsimd.dma_start(xt[:], x_tiled[mt])
                xt_tiles[mt] = xt

            def quant_tile(mt, use_act):
                # use_act: prologue mode — spread the big ops across ACT+DVE for
                # minimum wall-clock. Steady state keeps ACT free for rescales:
                # everything runs on DVE so the quant chain's serial latency
                # never sits ahead of PSUM-draining rescales in the ACT FIFO.
                c, mi = chunk_of(mt)
                load_tile(mt)
                load_tile(mt + 1)
                load_tile(mt + 2)
                xt = xt_tiles.pop(mt)

                amax = stat.tile([P, 1], F32, tag="amax", name="amax")
                nc.vector.tensor_reduce(
                    amax[:], xt[:], axis=mybir.AxisListType.X,
                    op=mybir.AluOpType.max, apply_absolute_value=True,
                )
                nc.vector.tensor_scalar_max(amax[:], amax[:], 1e-5)
                q = stat.tile([P, 1], F32, tag="q", name="q")
                nc.vector.reciprocal(q[:], amax[:])
                s_bf = stat.tile([P, 1], BF16, tag="s_bf", name="s_bf")
                nc.vector.tensor_scalar_mul(s_bf[:], q[:], 127.0)
                s_f32 = stat.tile([P, 1], F32, tag="s_f32", name="s_f32")
                nc.vector.tensor_copy(s_f32[:], s_bf[:])
                nc.vector.reciprocal(r_all[:, mt : mt + 1], s_f32[:])

                t = qtmp.tile([P, K], BF16, tag="t", name="t")
                xq8 = qtmp.tile([P, K], I8, tag="xq8", name="xq8")
                if use_act:
                    nc.scalar.activation(
                        t[:], xt[:], mybir.ActivationFunctionType.Copy,
                        scale=s_f32[:, 0:1],
                    )
                    if mi % 2 == 0:
                        nc.vector.tensor_copy(xq8[:], t[:])
                    else:
                        nc.scalar.copy(xq8[:], t[:])
                else:
                    nc.vector.tensor_scalar_mul(t[:], xt[:], s_f32[:, 0:1])
                    nc.vector.tensor_copy(xq8[:], t[:])
                # SWDGE store with int8 -> bf16 cast
                nc.gpsimd.dma_start(xq_dram[c][mi * P : (mi + 1) * P, :], xq8[:])

            def transpose_chunk(c):
                tiles = []
                for kc in range(KC):
                    tt = xqT_pool.tile(
                        [P, MAX_CHUNK_MT * P], BF16, tag=f"xqT{kc}", name=f"xqT{kc}"
                    )
                    nc.sync.dma_start_transpose(
                        tt[:, : CHUNK_MTS[c] * P],
                        xq_dram[c][:, kc * P : (kc + 1) * P],
                    )
                    tiles.append(tt)
                return tiles

            def matmul_mtile(c, mi, xqT):
                mt = CHUNK_STARTS[c] + mi
                y_sb = yout.tile([P, N], BF16, tag="y_sb", name="y_sb")
                for nt in range(NTN):
                    ps = psum.tile([P, N_TILE], F32, tag="ps", name="ps")
                    for kc in range(KC):
                        nc.tensor.matmul(
                            ps[:],
                            xqT[kc][:, mi * P : (mi + 1) * P],
                            wt[:, kc, nt * N_TILE : (nt + 1) * N_TILE],
                            start=(kc == 0),
                            stop=(kc == KC - 1),
                        )
                    nc.scalar.activation(
                        y_sb[:, nt * N_TILE : (nt + 1) * N_TILE],
                        ps[:],
                        mybir.ActivationFunctionType.Copy,
                        scale=r_all[:, mt : mt + 1],
                    )
                nc.sync.dma_start(y_tiled[mt], y_sb[:])

            # 2-chunk-deep software pipeline:
            #   during chunk c matmuls: transposes of c+1 execute (quantized
            #   during c-1), quant of c+2 is interleaved per m-tile.
            quant_cursor = [0]

            def ensure_quant_through(mt_end, use_act=False):
                while quant_cursor[0] < min(mt_end, NT):
                    quant_tile(quant_cursor[0], use_act=use_act)
                    quant_cursor[0] += 1

            prologue_end = CHUNK_STARTS[1] + CHUNK_MTS[1] if CHUNKS >= 2 else NT
            ensure_quant_through(CHUNK_MTS[0], use_act=True)
            ensure_quant_through(prologue_end, use_act=False)
            xqT_map = {0: transpose_chunk(0)}
            for c in range(CHUNKS):
                if c + 1 < CHUNKS:
                    xqT_map[c + 1] = transpose_chunk(c + 1)
                # emit quant of chunk c+2 eagerly (DVE self-paces; its queue
                # holds nothing else in steady state)
                tgt_end = (
                    CHUNK_STARTS[c + 2] + CHUNK_MTS[c + 2] if c + 2 < CHUNKS else NT
                )
                ensure_quant_through(tgt_end)
                for mi in range(CHUNK_MTS[c]):
                    matmul_mtile(c, mi, xqT_map[c])
                del xqT_map[c]

    nc.compile()
    return nc


def unpack_wT(packed_weight: np.ndarray, weight_scale: np.ndarray) -> np.ndarray:
    planes = [((packed_weight >> (2 * i)) & 3) for i in range(4)]
    w = np.concatenate(planes, axis=0).astype(np.float32) - 1.0  # [N, K]
    ws = np.float32(weight_scale.reshape(-1)[0])
    wT = np.ascontiguousarray((w / ws).T).astype(ml_dtypes.bfloat16)  # [K, N]
    return wT


_CACHE = {}


def run(x: np.ndarray, packed_weight: np.ndarray, weight_scale: np.ndarray,
        trace: bool = False, replays: int = 1, fused_quant: bool = FUSED_QUANT,
        tmpdir=None):
    """x: [B, S, K] bf16 -> y [B, S, N] bf16 (full, unsharded)."""
    key = (replays, fused_quant)
    if key not in _CACHE:
        _CACHE[key] = build_kernel(replays, fused_quant)
    nc = _CACHE[key]

    B, S, D = x.shape
    M = B * S
    assert M == M_LOC * N_CORES and D == K
    wT = unpack_wT(packed_weight, weight_scale)
    shards = np.ascontiguousarray(np.asarray(x).reshape(N_CORES, M_LOC, K))
    in_maps = [{"x": shards[i], "wT": wT} for i in range(N_CORES)]
    res = bass_utils.run_bass_kernel_spmd(
        nc, in_maps, core_ids=list(range(N_CORES)), trace=trace, tmpdir=tmpdir
    )
    y = np.stack([res.results[i]["y"] for i in range(N_CORES)], axis=0)
    return y.reshape(B, S, N), res



def kernel(x, packed_weight, weight_scale):
    """Harness entrypoint: FULL inputs -> FULL output.

    x: [4, 8192, 2048] bf16; packed_weight: [512, 2048] uint8;
    weight_scale: [1] bf16.  Returns [4, 8192, 2048] bf16.
    Sharding: data-parallel over tokens across the 8 NeuronCores;
    the (host-unpacked) ternary weight is replicated.
    """
    x = np.asarray(x)
    packed_weight = np.asarray(packed_weight)
    weight_scale = np.asarray(weight_scale)
    y, _ = run(x, packed_weight, weight_scale)
    return y

